# revision 32
# baseline (speedup 1.0000x reference)
"""MoE routing kernel (nn_MoE_12051678233096) for 8 TRN2 NeuronCores.

Computation (per reference):
    h = x @ w1            # [N,1024] @ [1024, 64*32] -> [N, 2048]
    z = keep top-4 of each group of 32 in h, zero the rest
    y = z @ w2            # [N, 2048] @ [2048, 1024]

Strategy: data-parallel over tokens (N=16384 -> 2048 per core), weights
replicated.  Per core, 16 token-tiles of 128 run through a 3-stage
software pipeline.  The DVE top-4 selection is the critical engine
(zero-gap ~247us wall; PE ~236us busy), so stage B batches TWO tiles
per op chain (128 "groups" per op) to amortize the ~150ns/op DVE
overhead, with the FIRST and LAST tiles selected singly so the
pipeline head (B needs its whole batch) and tail (one D after the
last chain) stay short.
  A: mm1 in fp16 (x and w1 pre-split/transposed on host; fp16 product
     error only flips top-4 selections at near-ties, rel-err ~1.6e-2
     < 2e-2 gate).  PE + scalar PSUM->SBUF copies (h f32 for the
     selection path + h16 twin for the value path).
  B: exact top-4 threshold per group of 32 via a bitonic partial-merge
     network on the DVE (f32 throughout; f16 thresholds are NOT safe:
     f16 ties at the rank-4/5 boundary add ~1e-2 error and any
     broadcast operand forces 1x rate anyway), then mask = (h >= t)
     and zb = h16 * mask (f16, 2x rate).
  D: zb transposed via PE (identity matmul, ldweights fully hidden),
     mm2 in fp16 quad-interleaved with the transposes so the last
     tile's mm2 chases the DVE chain, f16 output DMA (host upcasts;
     no measurable error change).

Startup: weights ride the gpsimd DMA ring (the Act queue must stay
free for h-copies; DVE can't DMA), w1 in half-column-group chunks so
tile 0's mm1 starts after 1/8 of w1 lands, PE pre-warmed to full
p-state via dummy matmuls on a memzero'd tile (no DMA dependency),
and B0's L1 level is emitted per-PSUM-bank quarter so the DVE starts
right after bank 0's h-copy.

Measured (8 cores, full clock): ~289.2us vs 317.8us baseline
(DVE wall verified at 1.04ns/elem f32 everywhere - the silicon rate);
note ~20% whole-chip clock-throttle episodes on shared devices make
single-run timings unreliable - compare min-of-several.

fp8 notes (measured): fp8e4+DoubleRow matmul = 216ns for out[128,512]
covering 2 k-chunks = exactly 2x fp16 (cost model's 0.5 cyc/row = 4x
is wrong on HW), so 3-term fp8 hi/lo schemes LOSE to fp16; single-term
fp8 fails the error budget (z/w2 quantization ~2-3% -> ~4e-2 rel-err).
"""

import numpy as np

import concourse.bass as bass
import concourse.mybir as mybir
import concourse.tile as tile
from concourse.bass_utils import run_bass_kernel_spmd
from concourse.vector_clock import ScopedClock

F32 = mybir.dt.float32
F16 = mybir.dt.float16
MAX = mybir.AluOpType.max
MIN = mybir.AluOpType.min
AF = mybir.ActivationFunctionType

N_CORES = 8
TOK_PER_CORE = 2048
N_TILES = 16  # of 128 tokens each
IN_DIM = 1024
PE_DIM = 2048  # 64 groups x 32 experts
OUT_DIM = 1024
# mm1 runs at scale 2^13 in PSUM (x*2^6 @ w1*2^7); h-copy applies 2^-13.
HSCALE = float(2.0**-13)


class _TC(tile.TileContext):
    """TileContext that legalizes sem waits to one per instruction
    (this walrus build rejects >1 sync wait on any instruction)."""

    def _lower_ordered_insts(self, ordered):
        for bb_name, insts in ordered.items():
            new_list = []
            for inst in insts:
                si = inst.sync_info
                if si is not None and len(si.on_wait) > 1:
                    waits = list(si.on_wait)
                    for w in waits[:-1]:
                        nop = mybir.InstNoOp(
                            name=f"waitsplit-{self.nc.next_id()}",
                            sync_info=mybir.SyncInfo(on_wait=[w], on_update=[]),
                            bass_nofuse=True,
                            engine=inst.engine,
                        )
                        new_list.append(nop)
                    inst.sync_info = mybir.SyncInfo(
                        on_wait=[waits[-1]], on_update=list(si.on_update)
                    )
                new_list.append(inst)
            ordered[bb_name] = new_list
        return super()._lower_ordered_insts(ordered)

    def _drain_and_barrier(self, tick_clock, wait_clock):
        import bass_rust

        nop_inst = self.nc.sync.nop(nofuse=True, hint="final_drain_waits")
        wait_clock.add_sem_waits(
            nop_inst.ins, ScopedClock({None: tick_clock.global_clock})
        )
        si = nop_inst.ins.sync_info
        waits = list(si.on_wait) if si is not None else []
        if len(waits) > 1:
            nop_inst.ins.sync_info = bass_rust.SyncInfo(
                on_wait=[waits[0]], on_update=list(si.on_update)
            )
            for w in waits[1:]:
                extra = self.nc.sync.nop(nofuse=True, hint="final_drain_waits")
                extra.ins.sync_info = bass_rust.SyncInfo(on_wait=[w], on_update=[])
        self.nc.sync.drain()
        # lean drain: sem-only barrier (full multi_engine_barrier costs ~2us
        # more), and no post-clear barrier - the gpsimd sem clears are the
        # last instructions, nothing follows that could race them
        self.nc.all_engine_barrier(sem_only=True)
        assert self.sems is not None
        popped = self.nc._tile_sem_poison_stack.pop()
        assert popped is self._sem_poison
        self.nc.clear_and_free_semaphores(list(self.sems.allocated().values()))


def _emit_topk_threshold(nc, tk, h_sb, m4, G, split_l1=False):
    """Emit DVE ops computing m4 = 4th-largest-of-each-32-group of h_sb.

    h_sb: [128, G, 32] f32 tile view; m4: [128, G, 1] f32 tile view.
    Exact bitonic partial-merge selection network (validated in numpy).
    G = number of groups processed at once (64 per token tile).
    split_l1: emit L1 in 16-group quarters so the chain can start as soon
    as the first PSUM bank's h-copy lands (used for the pipeline head)."""
    tt = nc.vector.tensor_tensor

    M = tk.tile([128, 128, 2, 16], F32, tag="tkM", name="tkM")[:, :G]
    # L1: fold halves -> 16 sorted 2-lists (row0=max, row1=min)
    if split_l1:
        for q in range(0, G, 16):
            qs = slice(q, q + 16)
            tt(M[:, qs, 0, :], h_sb[:, qs, 0:16], h_sb[:, qs, 16:32], op=MAX)
            tt(M[:, qs, 1, :], h_sb[:, qs, 0:16], h_sb[:, qs, 16:32], op=MIN)
    else:
        tt(M[:, :, 0, :], h_sb[:, :, 0:16], h_sb[:, :, 16:32], op=MAX)
        tt(M[:, :, 1, :], h_sb[:, :, 0:16], h_sb[:, :, 16:32], op=MIN)

    # L2: Batcher-merge 2-lists (j, j+8) -> 8 sorted 4-lists in T rows S0..S3
    T = tk.tile([128, 128, 4, 8], F32, tag="tkT", name="tkT")[:, :G]
    Q = tk.tile([128, 128, 8], F32, tag="tkQ", name="tkQ")[:, :G]
    R = tk.tile([128, 128, 8], F32, tag="tkR", name="tkR")[:, :G]
    tt(T[:, :, 0, :], M[:, :, 0, 0:8], M[:, :, 0, 8:16], op=MAX)
    tt(R[:], M[:, :, 1, 0:8], M[:, :, 1, 8:16], op=MAX)
    tt(Q[:], M[:, :, 0, 0:8], M[:, :, 0, 8:16], op=MIN)
    tt(T[:, :, 3, :], M[:, :, 1, 0:8], M[:, :, 1, 8:16], op=MIN)
    tt(T[:, :, 1, :], Q[:], R[:], op=MAX)
    tt(T[:, :, 2, :], Q[:], R[:], op=MIN)

    # L3/L4: merge sorted-4 list pairs, keep top-4, re-sort (bitonic)
    def merge_level(Tin, w, Uo, Vo, To):
        half = w // 2
        tt(Uo[:], Tin[:, :, :, 0:half], Tin[:, :, ::-1, half:w], op=MAX)
        tt(Vo[:, :, 0:2, :], Uo[:, :, 0:2, :], Uo[:, :, 2:4, :], op=MAX)
        tt(Vo[:, :, 2:4, :], Uo[:, :, 0:2, :], Uo[:, :, 2:4, :], op=MIN)
        tt(To[:, :, 0::2, :], Vo[:, :, 0::2, :], Vo[:, :, 1::2, :], op=MAX)
        tt(To[:, :, 1::2, :], Vo[:, :, 0::2, :], Vo[:, :, 1::2, :], op=MIN)

    # Aggressive buffer aliasing: later (smaller) levels reuse dead regions
    # of earlier buffers so the whole network fits in M, T, Q, R, U + m4.
    U = tk.tile([128, 128, 4, 4], F32, tag="tkU", name="tkU")[:, :G]
    V = T[:, :, :, 0:4]          # T dead after the first merge's U op
    T2 = U[:]                    # U dead once V is built
    merge_level(T[:], 8, U[:], V, T2)

    U2 = Q[:].rearrange("p g (r w) -> p g r w", r=4)   # Q dead after L2
    V2 = R[:].rearrange("p g (r w) -> p g r w", r=4)   # R dead after L2
    Mf = M[:].rearrange("p g r w -> p g (r w)")        # M dead after L2
    T3 = Mf[:, :, 0:8].rearrange("p g (r w) -> p g r w", r=4)
    merge_level(T2, 4, U2, V2, T3)

    # L5: final merge; min of the top-4 multiset = threshold
    U3 = Mf[:, :, 8:12].rearrange("p g (r w) -> p g r w", r=4)
    r2 = Mf[:, :, 12:14].rearrange("p g (r w) -> p g r w", r=2)
    tt(U3, T3[:, :, :, 0:1], T3[:, :, ::-1, 1:2], op=MAX)
    tt(r2, U3[:, :, 0:2, :], U3[:, :, 2:4, :], op=MIN)
    tt(m4[:], r2[:, :, 0, :], r2[:, :, 1, :], op=MIN)


def _build_nc():
    nc = bass.Bass("TRN2", target_bir_lowering=False, debug=False, num_devices=N_CORES)
    # x arrives host-transposed, fp16-scaled, tile-major (layout choice is
    # part of the sharding strategy): xth[t, p, k, j] = f16(64*x[t*128+p, k*128+...])
    xth_d = nc.dram_tensor("xth", [N_TILES, 128, 8, 128], F16, kind="ExternalInput")
    w1h_d = nc.dram_tensor("w1h", [128, 8, PE_DIM], F16, kind="ExternalInput")
    w2h_d = nc.dram_tensor("w2h", [128, 16, OUT_DIM], F16, kind="ExternalInput")
    id_d = nc.dram_tensor("ident", [128, 128], F16, kind="ExternalInput")
    y_d = nc.dram_tensor("y", [TOK_PER_CORE, OUT_DIM], F16, kind="ExternalOutput")

    A = mybir.AluOpType
    with _TC(nc) as tc:
        with (
            tc.tile_pool(name="weights", bufs=1) as wp,
            tc.tile_pool(name="xp", bufs=3) as xp,
            tc.tile_pool(name="hp", bufs=2) as hp,
            tc.tile_pool(name="h16p", bufs=2) as h16p,
            tc.tile_pool(name="tk", bufs=1) as tk,
            tc.tile_pool(name="zp", bufs=2) as zp,
            tc.tile_pool(name="ztp", bufs=2) as ztp,
            tc.tile_pool(name="op", bufs=2) as op,
            tc.tile_pool(name="psh", bufs=3, space="PSUM") as psh,
            tc.tile_pool(name="pwm", bufs=1, space="PSUM") as pwm,
            tc.tile_pool(name="pstr", bufs=2, space="PSUM") as pstr,
            tc.tile_pool(name="pso", bufs=2, space="PSUM") as pso,
        ):
            w1h = wp.tile([128, 8, PE_DIM], F16, tag="w1h")
            w2h = wp.tile([128, 16, OUT_DIM], F16, tag="w2h")
            ident = wp.tile([128, 128], F16, tag="ident")
            # weights go on the gpsimd HWDGE ring: the scalar (Act)
            # queue must stay free for the h-copies (a DMA_DIRECT2D on the
            # Act queue blocks them for ~us), and the sync ring carries the
            # x tiles.  DVE/GpSimd are idle at the head anyway.
            # w1 in half-column-group chunks (0.5MB): mm1 bank n's k=0..3
            # matmuls can start as soon as the first half of column group n
            # lands (first-transfer latency on a cold ring is ~8-12us).
            # ident rides after the first column group: the PE warm-up uses a
            # memzero'd tile, so ident is only needed by stage D's transposes.
            for n in range(4):
                cs = slice(n * 512, (n + 1) * 512)
                for kk in (0, 4):
                    nc.gpsimd.dma_start(
                        w1h[:, kk : kk + 4, cs], w1h_d[:, kk : kk + 4, cs]
                    )
                if n == 0:
                    nc.gpsimd.dma_start(ident[:], id_d[:])
            for c in range(4):
                nc.gpsimd.dma_start(
                    w2h[:, 4 * c : 4 * c + 4, :], w2h_d[:, 4 * c : 4 * c + 4, :]
                )

            # scratch PSUM target for HAM keep-warm dummy matmuls
            warm_ps = pwm.tile([128, 512], F32, tag="warm")
            # warm source is a zeroed SBUF tile (memzero on the idle Act
            # queue) so PE ramping starts ~7us in, without waiting for the
            # ident DMA to land
            wsrc = wp.tile([128, 128], F16, tag="wsrc")
            nc.scalar.memzero(wsrc[:])

            def keepwarm(n_mm):
                """Dummy matmuls to ramp the PE clock-gate to 8/8."""
                for r in range(n_mm):
                    nc.tensor.matmul(
                        warm_ps[:, 0:128], wsrc[:], wsrc[:],
                        start=(r == 0), stop=(r == n_mm - 1),
                    )

            # ramp the PE p-state while the first x/w1 DMAs land; sized to
            # keep the PE busy until w1's first chunk arrives (~15-20us) so
            # tile 0's mm1 runs at full clock
            keepwarm(52)

            # stage B batches two tiles per op chain (G=128), except the
            # FIRST and LAST tiles which are selected singly so the pipeline
            # head (B needs its whole batch) and tail stay short.  Buffer
            # b(t) = (t+1)//2, half (t+1)%2: tile 0 -> buf 0 half 1,
            # tiles (2p-1, 2p) -> buf p, tile 15 -> buf 8 half 0.
            NP = N_TILES // 2 + 1
            hq = [None] * NP
            h16q = [None] * NP
            zq = [None] * NP

            def stage_a(t):
                """x DMA + fp16 mm1 + PSUM->SBUF h (f32) and h16 copies
                into half (t+1)%2 of buffer (t+1)//2."""
                p, half = divmod(t + 1, 2)
                xt = xp.tile([128, 8, 128], F16, tag="xt", name=f"xt{t}")
                nc.sync.dma_start(xt[:], xth_d[t])
                if half == 0 or t == 0:
                    hq[p] = hp.tile([128, 128, 32], F32, tag="h", name=f"h{p}")
                    h16q[p] = h16p.tile([128, 128, 32], F16, tag="h16", name=f"h16{p}")
                h_sb = hq[p]
                h16 = h16q[p]
                off = half * 64
                # h16 copies trail the h f32 copies by one bank on the Act
                # queue: the B-chain's first ops wait on h f32 only, so this
                # gets each bank's h to the DVE ~0.7us sooner.
                hps_q = []
                for n in range(4):
                    hps = psh.tile([128, 512], F32, tag="hps", name="hps")
                    ncol = slice(n * 512, (n + 1) * 512)
                    for k in range(8):
                        nc.tensor.matmul(
                            hps[:], xt[:, k, :], w1h[:, k, ncol],
                            start=(k == 0), stop=(k == 7),
                        )
                    gsl = slice(off + n * 16, off + (n + 1) * 16)
                    nc.scalar.activation(h_sb[:, gsl, :], hps[:], AF.Copy, scale=HSCALE)
                    hps_q.append((hps, gsl))
                    if n >= 1:
                        ph, pg = hps_q[n - 1]
                        nc.scalar.activation(h16[:, pg, :], ph[:], AF.Copy, scale=HSCALE)
                ph, pg = hps_q[3]
                nc.scalar.activation(h16[:, pg, :], ph[:], AF.Copy, scale=HSCALE)

            def stage_b(p, lo=0, hi=128, split_l1=False, split_zb=False):
                """DVE: bitonic top-4 threshold, f16 mask, zb = h16*mask.
                Processes groups [lo:hi] of buffer p in one op chain."""
                G = hi - lo
                m4 = tk.tile([128, 128, 1], F32, tag="tkm4", name="tkm4")[:, :G]
                h_sb = hq[p][:, lo:hi]
                h16 = h16q[p][:, lo:hi]
                _emit_topk_threshold(nc, tk, h_sb, m4, G, split_l1=split_l1)
                mask = tk.tile([128, 128, 32], F16, tag="tkmask", name="tkmask")[:, :G]
                m4b = m4[:, :, 0].to_broadcast((128, G, 32))
                nc.vector.tensor_tensor(mask[:], h_sb[:], m4b, op=A.is_ge)
                if lo == 0 and hi == 128:
                    zb = zp.tile([128, 128, 32], F16, tag="zb", name=f"zb{p}")
                    zq[p] = zb
                elif zq[p] is None:
                    zb = zp.tile([128, 128, 32], F16, tag="zb", name=f"zb{p}")
                    zq[p] = zb
                else:
                    zb = zq[p]
                if split_zb:
                    # quartered so the last tile's transposes/mm2 can chase
                    # the DVE chain instead of waiting for all 64 groups
                    for q in range(0, G, 16):
                        qs = slice(q, q + 16)
                        nc.vector.tensor_tensor(
                            zb[:, lo + q : lo + q + 16], h16[:, qs], mask[:, qs],
                            op=A.mult,
                        )
                else:
                    nc.vector.tensor_tensor(zb[:, lo:hi], h16[:], mask[:], op=A.mult)

            def stage_d(t):
                """z transpose (PE), mm2, f16 output DMA."""
                p, half = divmod(t + 1, 2)
                rows = slice(t * 128, (t + 1) * 128)
                zT = ztp.tile([128, PE_DIM], F16, tag="zT", name=f"zT{t}")
                zbf = zq[p][:, half * 64 : half * 64 + 64, :].rearrange(
                    "p g e -> p (g e)"
                )
                out_sb = op.tile([128, OUT_DIM], F16, tag="outsb", name=f"o{t}")
                obank = [
                    pso.tile([128, 512], F32, tag="ops", name=f"ops{no}")
                    for no in range(2)
                ]
                # quad-pipelined: quad q's matmuls are emitted after quad
                # q+1's transposes, so the Act zT-copy round-trip (~1us)
                # hides behind PE work instead of stalling the mm2 chain;
                # mm2 still chases the transposes (and, for the last tiles,
                # the DVE chain)
                def tr_quad(quad):
                    pt = pstr.tile([128, 512], F16, tag="tr", name="pt")
                    for q in range(4):
                        k = quad * 4 + q
                        nc.tensor.transpose(
                            pt[:, q * 128 : (q + 1) * 128],
                            zbf[:, k * 128 : (k + 1) * 128],
                            ident[:],
                        )
                    nc.scalar.copy(zT[:, quad * 512 : (quad + 1) * 512], pt[:])

                def mm_quad(quad):
                    for no in range(2):
                        ocol = slice(no * 512, (no + 1) * 512)
                        for q in range(4):
                            k = quad * 4 + q
                            kc = slice(k * 128, (k + 1) * 128)
                            nc.tensor.matmul(
                                obank[no][:], zT[:, kc], w2h[:, k, ocol],
                                start=(k == 0), stop=(k == 15),
                            )

                tr_quad(0)
                for quad in range(1, 4):
                    tr_quad(quad)
                    mm_quad(quad - 1)
                mm_quad(3)
                for no in range(2):
                    ocol = slice(no * 512, (no + 1) * 512)
                    nc.scalar.copy(out_sb[:, ocol], obank[no][:])
                    nc.sync.dma_start(y_d[rows, ocol], out_sb[:, ocol])

            # software pipeline: single B0 (tile 0) so the DVE starts after
            # ONE mm1, pairs (1,2)..(13,14), single B8 (tile 15) so the tail
            # after the last DVE chain is one stage_d.
            stage_a(0)
            stage_b(0, lo=64, hi=128, split_l1=True)
            for p in range(1, 8):
                stage_a(2 * p - 1)
                stage_a(2 * p)
                # last pair: quarter the zb mults so D13/D14 can chase
                stage_b(p, split_zb=(p == 7))
                if p >= 2:
                    for t in (2 * p - 5, 2 * p - 4):
                        if t >= 0:
                            stage_d(t)
            stage_a(15)
            stage_b(8, lo=0, hi=64, split_zb=True)
            stage_d(11)
            stage_d(12)
            stage_d(13)
            stage_d(14)
            stage_d(15)

    return nc


def _prep_inputs(x, w1, w2):
    """Host-side shard + precision-split; returns per-core input maps."""
    x = np.ascontiguousarray(np.asarray(x), dtype=np.float32)
    w1f = np.asarray(w1, dtype=np.float32).reshape(IN_DIM, PE_DIM)
    w2f = np.asarray(w2, dtype=np.float32).reshape(PE_DIM, OUT_DIM)
    xf = x.reshape(-1, IN_DIM)
    assert xf.shape[0] == N_CORES * TOK_PER_CORE

    w1h = np.ascontiguousarray(
        (w1f * 128.0).astype(np.float16).reshape(8, 128, PE_DIM).transpose(1, 0, 2)
    )
    w2h = np.ascontiguousarray(
        w2f.astype(np.float16).reshape(16, 128, OUT_DIM).transpose(1, 0, 2)
    )
    ident = np.eye(128, dtype=np.float16)
    xhT = (xf * 64.0).astype(np.float16).T  # [IN_DIM, N]

    in_maps = []
    for i in range(N_CORES):
        seg = xhT[:, i * TOK_PER_CORE : (i + 1) * TOK_PER_CORE]
        xth = np.ascontiguousarray(
            seg.reshape(8, 128, N_TILES, 128).transpose(2, 1, 0, 3)
        )
        in_maps.append({"xth": xth, "w1h": w1h, "w2h": w2h, "ident": ident})
    return in_maps


_NC_CACHE = None


def kernel(x, w1, w2, top_k):
    global _NC_CACHE
    assert int(top_k) == 4
    lead_shape = np.asarray(x).shape[:-1]

    if _NC_CACHE is None:
        _NC_CACHE = _build_nc()
    nc = _NC_CACHE

    in_maps = _prep_inputs(x, w1, w2)
    res = run_bass_kernel_spmd(nc, in_maps, list(range(N_CORES)))
    out = np.concatenate([res.results[i]["y"] for i in range(N_CORES)], axis=0)
    return out.reshape(*lead_shape, OUT_DIM).astype(np.float32)


# revision 33
# speedup vs baseline: 1.1881x; 1.1881x over previous
"""MoE routing kernel (nn_MoE_12051678233096) for 8 TRN2 NeuronCores.

Computation (per reference):
    h = x @ w1            # [N,1024] @ [1024, 64*32] -> [N, 2048]
    z = keep top-4 of each group of 32 in h, zero the rest
    y = z @ w2            # [N, 2048] @ [2048, 1024]

Strategy: data-parallel over tokens (N=16384 -> 2048 per core), weights
replicated.  Per core, 16 token-tiles of 128 run through a 3-stage
software pipeline.  The DVE top-4 selection is the critical engine
(zero-gap ~247us wall; PE ~236us busy), so stage B batches TWO tiles
per op chain (128 "groups" per op) to amortize the ~150ns/op DVE
overhead, with the FIRST and LAST tiles selected singly so the
pipeline head (B needs its whole batch) and tail (one D after the
last chain) stay short.
  A: mm1 in fp16 (x and w1 pre-split/transposed on host; fp16 product
     error only flips top-4 selections at near-ties, rel-err ~1.6e-2
     < 2e-2 gate).  PE + scalar PSUM->SBUF copies (h f32 for the
     selection path + h16 twin for the value path).
  B: exact top-4 threshold per group of 32 via a bitonic partial-merge
     network on the DVE (f32 throughout; f16 thresholds are NOT safe:
     f16 ties at the rank-4/5 boundary add ~1e-2 error and any
     broadcast operand forces 1x rate anyway), then mask = (h >= t)
     and zb = h16 * mask (f16, 2x rate).
  D: zb transposed via PE (identity matmul, ldweights fully hidden),
     mm2 in fp16 quad-interleaved with the transposes so the last
     tile's mm2 chases the DVE chain, f16 output DMA (host upcasts;
     no measurable error change).

Startup: weights ride the gpsimd DMA ring (the Act queue must stay
free for h-copies; DVE can't DMA), w1 in half-column-group chunks so
tile 0's mm1 starts after 1/8 of w1 lands, PE pre-warmed to full
p-state via dummy matmuls on a memzero'd tile (no DMA dependency),
and B0's L1 level is emitted per-PSUM-bank quarter so the DVE starts
right after bank 0's h-copy.

Measured (8 cores, full clock): ~289.2us vs 317.8us baseline
(DVE wall verified at 1.04ns/elem f32 everywhere - the silicon rate);
note ~20% whole-chip clock-throttle episodes on shared devices make
single-run timings unreliable - compare min-of-several.

fp8 notes (measured): fp8e4+DoubleRow matmul = 216ns for out[128,512]
covering 2 k-chunks = exactly 2x fp16 (cost model's 0.5 cyc/row = 4x
is wrong on HW), so 3-term fp8 hi/lo schemes LOSE to fp16; single-term
fp8 fails the error budget (z/w2 quantization ~2-3% -> ~4e-2 rel-err).
"""

import numpy as np

import concourse.bass as bass
import concourse.mybir as mybir
import concourse.tile as tile
from concourse.bass_utils import run_bass_kernel_spmd
from concourse.vector_clock import ScopedClock

F32 = mybir.dt.float32
F16 = mybir.dt.float16
MAX = mybir.AluOpType.max
MIN = mybir.AluOpType.min
AF = mybir.ActivationFunctionType

N_CORES = 8
TOK_PER_CORE = 2048
N_TILES = 16  # of 128 tokens each
IN_DIM = 1024
PE_DIM = 2048  # 64 groups x 32 experts
OUT_DIM = 1024
# mm1 runs at scale 2^13 in PSUM (x*2^6 @ w1*2^7); h-copy applies 2^-13.
HSCALE = float(2.0**-13)


class _TC(tile.TileContext):
    """TileContext that legalizes sem waits to one per instruction
    (this walrus build rejects >1 sync wait on any instruction)."""

    def _lower_ordered_insts(self, ordered):
        for bb_name, insts in ordered.items():
            new_list = []
            for inst in insts:
                si = inst.sync_info
                if si is not None and len(si.on_wait) > 1:
                    waits = list(si.on_wait)
                    for w in waits[:-1]:
                        nop = mybir.InstNoOp(
                            name=f"waitsplit-{self.nc.next_id()}",
                            sync_info=mybir.SyncInfo(on_wait=[w], on_update=[]),
                            bass_nofuse=True,
                            engine=inst.engine,
                        )
                        new_list.append(nop)
                    inst.sync_info = mybir.SyncInfo(
                        on_wait=[waits[-1]], on_update=list(si.on_update)
                    )
                new_list.append(inst)
            ordered[bb_name] = new_list
        return super()._lower_ordered_insts(ordered)

    def _drain_and_barrier(self, tick_clock, wait_clock):
        import bass_rust

        nop_inst = self.nc.sync.nop(nofuse=True, hint="final_drain_waits")
        wait_clock.add_sem_waits(
            nop_inst.ins, ScopedClock({None: tick_clock.global_clock})
        )
        si = nop_inst.ins.sync_info
        waits = list(si.on_wait) if si is not None else []
        if len(waits) > 1:
            nop_inst.ins.sync_info = bass_rust.SyncInfo(
                on_wait=[waits[0]], on_update=list(si.on_update)
            )
            for w in waits[1:]:
                extra = self.nc.sync.nop(nofuse=True, hint="final_drain_waits")
                extra.ins.sync_info = bass_rust.SyncInfo(on_wait=[w], on_update=[])
        self.nc.sync.drain()
        # lean drain: sem-only barrier (full multi_engine_barrier costs ~2us
        # more), and no post-clear barrier - the gpsimd sem clears are the
        # last instructions, nothing follows that could race them
        self.nc.all_engine_barrier(sem_only=True)
        assert self.sems is not None
        popped = self.nc._tile_sem_poison_stack.pop()
        assert popped is self._sem_poison
        self.nc.clear_and_free_semaphores(list(self.sems.allocated().values()))


def _emit_topk_threshold(nc, tk, h_sb, m4, G, split_l1=False):
    """Emit DVE ops computing m4 = 4th-largest-of-each-32-group of h_sb.

    h_sb: [128, G, 32] f32 tile view; m4: [128, G, 1] f32 tile view.
    Exact bitonic partial-merge selection network (validated in numpy).
    G = number of groups processed at once (64 per token tile).
    split_l1: emit L1 in 16-group quarters so the chain can start as soon
    as the first PSUM bank's h-copy lands (used for the pipeline head)."""
    tt = nc.vector.tensor_tensor

    M = tk.tile([128, 128, 2, 16], F32, tag="tkM", name="tkM")[:, :G]
    # L1: fold halves -> 16 sorted 2-lists (row0=max, row1=min)
    if split_l1:
        for q in range(0, G, 16):
            qs = slice(q, q + 16)
            tt(M[:, qs, 0, :], h_sb[:, qs, 0:16], h_sb[:, qs, 16:32], op=MAX)
            tt(M[:, qs, 1, :], h_sb[:, qs, 0:16], h_sb[:, qs, 16:32], op=MIN)
    else:
        tt(M[:, :, 0, :], h_sb[:, :, 0:16], h_sb[:, :, 16:32], op=MAX)
        tt(M[:, :, 1, :], h_sb[:, :, 0:16], h_sb[:, :, 16:32], op=MIN)

    # L2: Batcher-merge 2-lists (j, j+8) -> 8 sorted 4-lists in T rows S0..S3
    T = tk.tile([128, 128, 4, 8], F32, tag="tkT", name="tkT")[:, :G]
    Q = tk.tile([128, 128, 8], F32, tag="tkQ", name="tkQ")[:, :G]
    R = tk.tile([128, 128, 8], F32, tag="tkR", name="tkR")[:, :G]
    tt(T[:, :, 0, :], M[:, :, 0, 0:8], M[:, :, 0, 8:16], op=MAX)
    tt(R[:], M[:, :, 1, 0:8], M[:, :, 1, 8:16], op=MAX)
    tt(Q[:], M[:, :, 0, 0:8], M[:, :, 0, 8:16], op=MIN)
    tt(T[:, :, 3, :], M[:, :, 1, 0:8], M[:, :, 1, 8:16], op=MIN)
    tt(T[:, :, 1, :], Q[:], R[:], op=MAX)
    tt(T[:, :, 2, :], Q[:], R[:], op=MIN)

    # L3/L4: merge sorted-4 list pairs, keep top-4, re-sort (bitonic)
    def merge_level(Tin, w, Uo, Vo, To):
        half = w // 2
        tt(Uo[:], Tin[:, :, :, 0:half], Tin[:, :, ::-1, half:w], op=MAX)
        tt(Vo[:, :, 0:2, :], Uo[:, :, 0:2, :], Uo[:, :, 2:4, :], op=MAX)
        tt(Vo[:, :, 2:4, :], Uo[:, :, 0:2, :], Uo[:, :, 2:4, :], op=MIN)
        tt(To[:, :, 0::2, :], Vo[:, :, 0::2, :], Vo[:, :, 1::2, :], op=MAX)
        tt(To[:, :, 1::2, :], Vo[:, :, 0::2, :], Vo[:, :, 1::2, :], op=MIN)

    # Aggressive buffer aliasing: later (smaller) levels reuse dead regions
    # of earlier buffers so the whole network fits in M, T, Q, R, U + m4.
    U = tk.tile([128, 128, 4, 4], F32, tag="tkU", name="tkU")[:, :G]
    V = T[:, :, :, 0:4]          # T dead after the first merge's U op
    T2 = U[:]                    # U dead once V is built
    merge_level(T[:], 8, U[:], V, T2)

    U2 = Q[:].rearrange("p g (r w) -> p g r w", r=4)   # Q dead after L2
    V2 = R[:].rearrange("p g (r w) -> p g r w", r=4)   # R dead after L2
    Mf = M[:].rearrange("p g r w -> p g (r w)")        # M dead after L2
    T3 = Mf[:, :, 0:8].rearrange("p g (r w) -> p g r w", r=4)
    merge_level(T2, 4, U2, V2, T3)

    # L5: final merge; min of the top-4 multiset = threshold
    U3 = Mf[:, :, 8:12].rearrange("p g (r w) -> p g r w", r=4)
    r2 = Mf[:, :, 12:14].rearrange("p g (r w) -> p g r w", r=2)
    tt(U3, T3[:, :, :, 0:1], T3[:, :, ::-1, 1:2], op=MAX)
    tt(r2, U3[:, :, 0:2, :], U3[:, :, 2:4, :], op=MIN)
    tt(m4[:], r2[:, :, 0, :], r2[:, :, 1, :], op=MIN)


def _build_nc():
    nc = bass.Bass("TRN2", target_bir_lowering=False, debug=False, num_devices=N_CORES)
    # x arrives host-transposed, fp16-scaled, tile-major (layout choice is
    # part of the sharding strategy): xth[t, p, k, j] = f16(64*x[t*128+p, k*128+...])
    xth_d = nc.dram_tensor("xth", [N_TILES, 128, 8, 128], F16, kind="ExternalInput")
    w1h_d = nc.dram_tensor("w1h", [128, 8, PE_DIM], F16, kind="ExternalInput")
    w2h_d = nc.dram_tensor("w2h", [128, 16, OUT_DIM], F16, kind="ExternalInput")
    id_d = nc.dram_tensor("ident", [128, 128], F16, kind="ExternalInput")
    y_d = nc.dram_tensor("y", [TOK_PER_CORE, OUT_DIM], F16, kind="ExternalOutput")

    A = mybir.AluOpType
    with _TC(nc) as tc:
        with (
            tc.tile_pool(name="weights", bufs=1) as wp,
            tc.tile_pool(name="xp", bufs=3) as xp,
            tc.tile_pool(name="hp", bufs=2) as hp,
            tc.tile_pool(name="h16p", bufs=2) as h16p,
            tc.tile_pool(name="tk", bufs=1) as tk,
            tc.tile_pool(name="zp", bufs=2) as zp,
            tc.tile_pool(name="ztp", bufs=2) as ztp,
            tc.tile_pool(name="op", bufs=2) as op,
            tc.tile_pool(name="psh", bufs=3, space="PSUM") as psh,
            tc.tile_pool(name="pwm", bufs=1, space="PSUM") as pwm,
            tc.tile_pool(name="pstr", bufs=2, space="PSUM") as pstr,
            tc.tile_pool(name="pso", bufs=2, space="PSUM") as pso,
        ):
            w1h = wp.tile([128, 8, PE_DIM], F16, tag="w1h")
            w2h = wp.tile([128, 16, OUT_DIM], F16, tag="w2h")
            ident = wp.tile([128, 128], F16, tag="ident")
            # weights go on the gpsimd HWDGE ring: the scalar (Act)
            # queue must stay free for the h-copies (a DMA_DIRECT2D on the
            # Act queue blocks them for ~us), and the sync ring carries the
            # x tiles.  DVE/GpSimd are idle at the head anyway.
            # w1 in half-column-group chunks (0.5MB): mm1 bank n's k=0..3
            # matmuls can start as soon as the first half of column group n
            # lands (first-transfer latency on a cold ring is ~8-12us).
            # ident rides after the first column group: the PE warm-up uses a
            # memzero'd tile, so ident is only needed by stage D's transposes.
            # column group 0 rides the sync (SP/HWDGE) ring, which moves
            # ~3x faster from cold than the gpsimd SWDGE ring (measured
            # 256KB in 0.7us vs 1MB in ~9us) - it lands ~9us sooner, and
            # x-tile 0 behind it still arrives before mm1 needs it.
            for kk in (0, 4):
                nc.sync.dma_start(
                    w1h[:, kk : kk + 4, 0:512], w1h_d[:, kk : kk + 4, 0:512]
                )
            for n in range(1, 4):
                cs = slice(n * 512, (n + 1) * 512)
                for kk in (0, 4):
                    nc.gpsimd.dma_start(
                        w1h[:, kk : kk + 4, cs], w1h_d[:, kk : kk + 4, cs]
                    )
                if n == 1:
                    nc.gpsimd.dma_start(ident[:], id_d[:])
            for c in range(4):
                nc.gpsimd.dma_start(
                    w2h[:, 4 * c : 4 * c + 4, :], w2h_d[:, 4 * c : 4 * c + 4, :]
                )

            # scratch PSUM target for HAM keep-warm dummy matmuls
            warm_ps = pwm.tile([128, 512], F32, tag="warm")
            # warm source is a zeroed SBUF tile (memzero on the idle Act
            # queue) so PE ramping starts ~7us in, without waiting for the
            # ident DMA to land
            wsrc = wp.tile([128, 128], F16, tag="wsrc")
            nc.scalar.memzero(wsrc[:])

            def keepwarm(n_mm):
                """Dummy matmuls to ramp the PE clock-gate to 8/8."""
                for r in range(n_mm):
                    nc.tensor.matmul(
                        warm_ps[:, 0:128], wsrc[:], wsrc[:],
                        start=(r == 0), stop=(r == n_mm - 1),
                    )

            # ramp the PE p-state while the first x/w1 DMAs land; sized to
            # keep the PE busy until w1's first chunk arrives (~15-20us) so
            # tile 0's mm1 runs at full clock
            keepwarm(38)

            # stage B batches two tiles per op chain (G=128), except the
            # FIRST and LAST tiles which are selected singly so the pipeline
            # head (B needs its whole batch) and tail stay short.  Buffer
            # b(t) = (t+1)//2, half (t+1)%2: tile 0 -> buf 0 half 1,
            # tiles (2p-1, 2p) -> buf p, tile 15 -> buf 8 half 0.
            NP = N_TILES // 2 + 1
            hq = [None] * NP
            h16q = [None] * NP
            zq = [None] * NP

            def stage_a(t):
                """x DMA + fp16 mm1 + PSUM->SBUF h (f32) and h16 copies
                into half (t+1)%2 of buffer (t+1)//2."""
                p, half = divmod(t + 1, 2)
                xt = xp.tile([128, 8, 128], F16, tag="xt", name=f"xt{t}")
                nc.sync.dma_start(xt[:], xth_d[t])
                if half == 0 or t == 0:
                    hq[p] = hp.tile([128, 128, 32], F32, tag="h", name=f"h{p}")
                    h16q[p] = h16p.tile([128, 128, 32], F16, tag="h16", name=f"h16{p}")
                h_sb = hq[p]
                h16 = h16q[p]
                off = half * 64
                # h16 copies trail the h f32 copies by one bank on the Act
                # queue: the B-chain's first ops wait on h f32 only, so this
                # gets each bank's h to the DVE ~0.7us sooner.
                hps_q = []
                for n in range(4):
                    hps = psh.tile([128, 512], F32, tag="hps", name="hps")
                    ncol = slice(n * 512, (n + 1) * 512)
                    for k in range(8):
                        nc.tensor.matmul(
                            hps[:], xt[:, k, :], w1h[:, k, ncol],
                            start=(k == 0), stop=(k == 7),
                        )
                    gsl = slice(off + n * 16, off + (n + 1) * 16)
                    nc.scalar.activation(h_sb[:, gsl, :], hps[:], AF.Copy, scale=HSCALE)
                    hps_q.append((hps, gsl))
                    if n >= 1:
                        ph, pg = hps_q[n - 1]
                        nc.scalar.activation(h16[:, pg, :], ph[:], AF.Copy, scale=HSCALE)
                ph, pg = hps_q[3]
                nc.scalar.activation(h16[:, pg, :], ph[:], AF.Copy, scale=HSCALE)

            def stage_b(p, lo=0, hi=128, split_l1=False, split_zb=False):
                """DVE: bitonic top-4 threshold, f16 mask, zb = h16*mask.
                Processes groups [lo:hi] of buffer p in one op chain."""
                G = hi - lo
                m4 = tk.tile([128, 128, 1], F32, tag="tkm4", name="tkm4")[:, :G]
                h_sb = hq[p][:, lo:hi]
                h16 = h16q[p][:, lo:hi]
                _emit_topk_threshold(nc, tk, h_sb, m4, G, split_l1=split_l1)
                mask = tk.tile([128, 128, 32], F16, tag="tkmask", name="tkmask")[:, :G]
                m4b = m4[:, :, 0].to_broadcast((128, G, 32))
                nc.vector.tensor_tensor(mask[:], h_sb[:], m4b, op=A.is_ge)
                if lo == 0 and hi == 128:
                    zb = zp.tile([128, 128, 32], F16, tag="zb", name=f"zb{p}")
                    zq[p] = zb
                elif zq[p] is None:
                    zb = zp.tile([128, 128, 32], F16, tag="zb", name=f"zb{p}")
                    zq[p] = zb
                else:
                    zb = zq[p]
                if split_zb:
                    # quartered so the last tile's transposes/mm2 can chase
                    # the DVE chain instead of waiting for all 64 groups
                    for q in range(0, G, 16):
                        qs = slice(q, q + 16)
                        nc.vector.tensor_tensor(
                            zb[:, lo + q : lo + q + 16], h16[:, qs], mask[:, qs],
                            op=A.mult,
                        )
                else:
                    nc.vector.tensor_tensor(zb[:, lo:hi], h16[:], mask[:], op=A.mult)

            def stage_d(t):
                """z transpose (PE), mm2, f16 output DMA."""
                p, half = divmod(t + 1, 2)
                rows = slice(t * 128, (t + 1) * 128)
                zT = ztp.tile([128, PE_DIM], F16, tag="zT", name=f"zT{t}")
                zbf = zq[p][:, half * 64 : half * 64 + 64, :].rearrange(
                    "p g e -> p (g e)"
                )
                out_sb = op.tile([128, OUT_DIM], F16, tag="outsb", name=f"o{t}")
                obank = [
                    pso.tile([128, 512], F32, tag="ops", name=f"ops{no}")
                    for no in range(2)
                ]
                # quad-pipelined: quad q's matmuls are emitted after quad
                # q+1's transposes, so the Act zT-copy round-trip (~1us)
                # hides behind PE work instead of stalling the mm2 chain;
                # mm2 still chases the transposes (and, for the last tiles,
                # the DVE chain)
                def tr_quad(quad):
                    pt = pstr.tile([128, 512], F16, tag="tr", name="pt")
                    for q in range(4):
                        k = quad * 4 + q
                        nc.tensor.transpose(
                            pt[:, q * 128 : (q + 1) * 128],
                            zbf[:, k * 128 : (k + 1) * 128],
                            ident[:],
                        )
                    nc.scalar.copy(zT[:, quad * 512 : (quad + 1) * 512], pt[:])

                def mm_quad(quad):
                    for no in range(2):
                        ocol = slice(no * 512, (no + 1) * 512)
                        for q in range(4):
                            k = quad * 4 + q
                            kc = slice(k * 128, (k + 1) * 128)
                            nc.tensor.matmul(
                                obank[no][:], zT[:, kc], w2h[:, k, ocol],
                                start=(k == 0), stop=(k == 15),
                            )

                tr_quad(0)
                for quad in range(1, 4):
                    tr_quad(quad)
                    mm_quad(quad - 1)
                mm_quad(3)
                for no in range(2):
                    ocol = slice(no * 512, (no + 1) * 512)
                    nc.scalar.copy(out_sb[:, ocol], obank[no][:])
                    nc.sync.dma_start(y_d[rows, ocol], out_sb[:, ocol])

            # software pipeline: single B0 (tile 0) so the DVE starts after
            # ONE mm1, pairs (1,2)..(13,14), single B8 (tile 15) so the tail
            # after the last DVE chain is one stage_d.
            stage_a(0)
            stage_b(0, lo=64, hi=128, split_l1=True)
            for p in range(1, 8):
                stage_a(2 * p - 1)
                stage_a(2 * p)
                # last pair: quarter the zb mults so D13/D14 can chase
                stage_b(p, split_zb=(p == 7))
                if p >= 2:
                    for t in (2 * p - 5, 2 * p - 4):
                        if t >= 0:
                            stage_d(t)
            stage_a(15)
            stage_b(8, lo=0, hi=64, split_zb=True)
            stage_d(11)
            stage_d(12)
            stage_d(13)
            stage_d(14)
            stage_d(15)

    return nc


def _prep_inputs(x, w1, w2):
    """Host-side shard + precision-split; returns per-core input maps."""
    x = np.ascontiguousarray(np.asarray(x), dtype=np.float32)
    w1f = np.asarray(w1, dtype=np.float32).reshape(IN_DIM, PE_DIM)
    w2f = np.asarray(w2, dtype=np.float32).reshape(PE_DIM, OUT_DIM)
    xf = x.reshape(-1, IN_DIM)
    assert xf.shape[0] == N_CORES * TOK_PER_CORE

    w1h = np.ascontiguousarray(
        (w1f * 128.0).astype(np.float16).reshape(8, 128, PE_DIM).transpose(1, 0, 2)
    )
    w2h = np.ascontiguousarray(
        w2f.astype(np.float16).reshape(16, 128, OUT_DIM).transpose(1, 0, 2)
    )
    ident = np.eye(128, dtype=np.float16)
    xhT = (xf * 64.0).astype(np.float16).T  # [IN_DIM, N]

    in_maps = []
    for i in range(N_CORES):
        seg = xhT[:, i * TOK_PER_CORE : (i + 1) * TOK_PER_CORE]
        xth = np.ascontiguousarray(
            seg.reshape(8, 128, N_TILES, 128).transpose(2, 1, 0, 3)
        )
        in_maps.append({"xth": xth, "w1h": w1h, "w2h": w2h, "ident": ident})
    return in_maps


_NC_CACHE = None


def kernel(x, w1, w2, top_k):
    global _NC_CACHE
    assert int(top_k) == 4
    lead_shape = np.asarray(x).shape[:-1]

    if _NC_CACHE is None:
        _NC_CACHE = _build_nc()
    nc = _NC_CACHE

    in_maps = _prep_inputs(x, w1, w2)
    res = run_bass_kernel_spmd(nc, in_maps, list(range(N_CORES)))
    out = np.concatenate([res.results[i]["y"] for i in range(N_CORES)], axis=0)
    return out.reshape(*lead_shape, OUT_DIM).astype(np.float32)
